# revision 7
# baseline (speedup 1.0000x reference)
# Trainium2 Bass kernel for nn_MultiHeadAttention_71674414235938
#
# MHA with a cross-modal additive bias gathered from a 3x3 table and a causal
# mask, B=1, S=2048, HID=1024, H=16 heads of D=64.
#
# Sharding: tensor-parallel over heads. 2 heads per core (dq slice of 128).
# Each core computes q/k/v projections for its heads, head-local attention,
# and a partial output ctx_c @ Wo[:, c*128:(c+1)*128].T which the host sums.
#
# Device-side layout choices:
#   * scores are computed TRANSPOSED: sT[j, i] = k[j]·q[i] (j on partitions),
#     so softmax-denominators and the attn@V contraction both run without any
#     on-chip transposes:  ctxT[d, i] = sum_j v'[j, d] * attnT[j, i]  with
#     lhsT = v' (natural layout) and rhs = attnT (as produced).
#   * the 3x3 cross-modal bias is rank-3:  bias = (onehot(m) @ cmw) @ onehot(m).T
#     so it is folded into the scores matmul by appending 3 rows (U.T to the
#     q side, R.T to the k side), K = 64+3 = 67.
#   * softmax runs without max-subtraction: scores are O(+-6) here, exp is
#     safely in fp32 range.
#   * a ones-column appended to v makes the PE accumulate the softmax
#     denominator into ctxT row 64; normalization happens on the way out of
#     PSUM (reciprocal + partition-broadcast DMA + multiply).
#   * causal structure: score blocks entirely above the diagonal are skipped;
#     diagonal staircase blocks are masked multiplicatively after exp.

import math

import numpy as np
import ml_dtypes

B, S, HID, H, D = 1, 2048, 1024, 16, 64
NCORES = 8
HPC = H // NCORES          # heads per core = 2
DPC = HPC * D              # head-dim columns per core = 128
KC = HID // 128            # contraction chunks = 8
NIC = S // 512             # 512-wide i-chunks = 4
NJB = S // 128             # 128-tall j-blocks = 16

BF16 = ml_dtypes.bfloat16

_CACHE = {}


def _build(causal: bool, has_bq: bool, has_bk: bool, has_bv: bool):
    import concourse.bass as bass
    import concourse.bacc as bacc
    import concourse.mybir as mybir
    import concourse.tile as tile

    fp32 = mybir.dt.float32
    bf16 = mybir.dt.bfloat16
    Exp = mybir.ActivationFunctionType.Exp

    nc = bacc.Bacc()

    xT = nc.declare_dram_parameter("xT", [HID, S], bf16, isOutput=False)
    wqT = nc.declare_dram_parameter("wqT", [HID, DPC], bf16, isOutput=False)
    wkT = nc.declare_dram_parameter("wkT", [HID, DPC], bf16, isOutput=False)
    wvT = nc.declare_dram_parameter("wvT", [HID, DPC], bf16, isOutput=False)
    woT = nc.declare_dram_parameter("woT", [DPC, HID], bf16, isOutput=False)
    uT = nc.declare_dram_parameter("uT", [4, S], bf16, isOutput=False)
    rT = nc.declare_dram_parameter("rT", [4, S], bf16, isOutput=False)
    if has_bq:
        bq = nc.declare_dram_parameter("bq", [DPC, 1], fp32, isOutput=False)
    if has_bk:
        bk = nc.declare_dram_parameter("bk", [DPC, 1], fp32, isOutput=False)
    if has_bv:
        bv = nc.declare_dram_parameter("bv", [1, DPC], fp32, isOutput=False)
    if not causal:
        maskT = nc.declare_dram_parameter("maskT", [S, S], bf16, isOutput=False)
    out = nc.declare_dram_parameter("out", [S, HID], fp32, isOutput=True)

    with tile.TileContext(nc) as tc:
        with tc.tile_pool(name="persist", bufs=1) as pp:
            # persistent SBUF tensors
            wo_sb = pp.tile([128, HID], bf16)
            nc.sync.dma_start(out=wo_sb, in_=woT[:, :])
            # qU / kR: per head, 67 live rows ([0:64] proj, [64:67] bias factors)
            qU = [pp.tile([67, S], bf16, name=f"qU{h}") for h in range(HPC)]
            kR = [pp.tile([67, S], bf16, name=f"kR{h}") for h in range(HPC)]
            for h in range(HPC):
                nc.sync.dma_start(out=qU[h][64:67, :], in_=uT[0:3, :])
                nc.sync.dma_start(out=kR[h][64:67, :], in_=rT[0:3, :])
            # v' tiles: [j%128, jb, 65] with ones in column 64
            vp = [pp.tile([128, NJB, 65], bf16, name=f"vp{h}") for h in range(HPC)]
            for h in range(HPC):
                nc.vector.memset(vp[h][:, :, 64:65], 1.0)
            # normalized transposed context, both heads: [dc, i]
            ctxT = pp.tile([128, S], bf16)
            # staircase causal mask for the diagonal 128-col strip: the strip
            # at[:, d:d+128] covers global cols i = jb*128 + f against rows
            # j = jb*128 + p, so keep (1.0) exactly where f >= p
            stair = None
            if causal:
                stair = pp.tile([128, 128], bf16)
                nc.vector.memset(stair, 1.0)
                nc.gpsimd.affine_select(
                    out=stair, in_=stair,
                    compare_op=mybir.AluOpType.is_ge,
                    fill=0.0,
                    base=0,
                    pattern=[[1, 128]],
                    channel_multiplier=-1,
                )
            if has_bq:
                bq_sb = pp.tile([DPC, 1], fp32)
                nc.sync.dma_start(out=bq_sb, in_=bq[:, :])
            if has_bk:
                bk_sb = pp.tile([DPC, 1], fp32)
                nc.sync.dma_start(out=bk_sb, in_=bk[:, :])
            if has_bv:
                bv_sb = pp.tile([128, DPC], fp32)
                bv_ap = bv[:, :]
                nc.gpsimd.dma_start(
                    out=bv_sb,
                    in_=bass.AP(tensor=bv_ap.tensor, offset=bv_ap.offset,
                                ap=[[0, 128], bv_ap.ap[1]]),
                )

            # ---------------- phase 1: QKV projections ----------------
            with (
                tc.tile_pool(name="ph1", bufs=1) as p1,
                tc.tile_pool(name="ps1", bufs=1, space="PSUM") as ps1,
            ):
                xT_sb = p1.tile([128, KC, S], bf16)
                nc.sync.dma_start(
                    out=xT_sb, in_=xT[:, :].rearrange("(kc p) n -> p kc n", p=128)
                )
                w_sbs = {}
                for nm, src in (("q", wqT), ("k", wkT), ("v", wvT)):
                    w_sb = w_sbs[nm] = p1.tile(
                        [128, KC, DPC], bf16, name=f"w{nm}_sb"
                    )
                    nc.sync.dma_start(
                        out=w_sb, in_=src[:, :].rearrange("(kc p) m -> p kc m", p=128)
                    )

                # q, k transposed projections: out[dq, i], kc-outer so the
                # stationary operand is reused across the 4 i-chunks
                for nm, dsts, bias_sb in (
                    ("q", qU, bq_sb if has_bq else None),
                    ("k", kR, bk_sb if has_bk else None),
                ):
                    pss = [
                        ps1.tile([128, 512], fp32, tag=f"proj{n}", name=f"ps_{nm}{n}")
                        for n in range(NIC)
                    ]
                    for kc in range(KC):
                        for n in range(NIC):
                            nc.tensor.matmul(
                                pss[n],
                                lhsT=w_sbs[nm][:, kc, :],
                                rhs=xT_sb[:, kc, n * 512:(n + 1) * 512],
                                start=(kc == 0),
                                stop=(kc == KC - 1),
                            )
                    for n in range(NIC):
                        for h in range(HPC):
                            dst = dsts[h][0:64, n * 512:(n + 1) * 512]
                            src = pss[n][h * 64:(h + 1) * 64, :]
                            if bias_sb is not None:
                                nc.vector.tensor_scalar_add(
                                    dst, src, bias_sb[h * 64:(h + 1) * 64, 0:1]
                                )
                            else:
                                nc.vector.tensor_copy(dst, src)

                # v natural projection: out[j, dv] so no transpose is needed
                for jb in range(NJB):
                    psv = ps1.tile([128, DPC], fp32, tag="vproj", bufs=2)
                    for kc in range(KC):
                        nc.tensor.matmul(
                            psv,
                            lhsT=xT_sb[:, kc, jb * 128:(jb + 1) * 128],
                            rhs=w_sbs["v"][:, kc, :],
                            start=(kc == 0),
                            stop=(kc == KC - 1),
                        )
                    for h in range(HPC):
                        dst = vp[h][:, jb, 0:64]
                        src = psv[:, h * 64:(h + 1) * 64]
                        if has_bv:
                            nc.vector.tensor_add(
                                dst, src, bv_sb[:, h * 64:(h + 1) * 64]
                            )
                        else:
                            nc.vector.tensor_copy(dst, src)

            # ---------------- phase 2: attention per head ----------------
            with (
                tc.tile_pool(name="ph2", bufs=1) as p2,
                tc.tile_pool(name="ps2", bufs=1, space="PSUM") as ps2,
            ):
                for h in range(HPC):
                    at_tiles = []
                    for jb in range(NJB):
                        ics = (jb * 128) // 512 if causal else 0
                        w = S - ics * 512
                        sc = ps2.tile([128, 2048], fp32, tag="sc", name=f"sc{h}_{jb}")
                        for n in range(ics, NIC):
                            nc.tensor.matmul(
                                sc[:, (n - ics) * 512:(n - ics + 1) * 512],
                                lhsT=kR[h][:, jb * 128:(jb + 1) * 128],
                                rhs=qU[h][:, n * 512:(n + 1) * 512],
                                start=True,
                                stop=True,
                            )
                        at = p2.tile(
                            [128, w], bf16, tag=f"at{jb}", bufs=2, name=f"at{h}_{jb}"
                        )
                        nc.scalar.activation(at, sc[:, 0:w], Exp)
                        if causal:
                            d = (jb % 4) * 128
                            if d:
                                nc.gpsimd.memset(at[:, 0:d], 0.0)
                            nc.vector.tensor_mul(
                                at[:, d:d + 128], at[:, d:d + 128], stair
                            )
                        else:
                            mt = p2.tile([128, S], bf16, tag="mt", bufs=2, name=f"mt{h}_{jb}")
                            nc.sync.dma_start(
                                out=mt, in_=maskT[jb * 128:(jb + 1) * 128, :]
                            )
                            nc.vector.tensor_mul(at, at, mt)
                        at_tiles.append((at, ics))

                    for ic in range(NIC):
                        jmax = (ic + 1) * 4 if causal else NJB
                        cps = ps2.tile([65, 512], fp32, tag="ctx", bufs=2, name=f"cps{h}_{ic}")
                        for jb in range(jmax):
                            at, ics = at_tiles[jb]
                            nc.tensor.matmul(
                                cps,
                                lhsT=vp[h][:, jb, :],
                                rhs=at[:, (ic - ics) * 512:(ic - ics + 1) * 512],
                                start=(jb == 0),
                                stop=(jb == jmax - 1),
                            )
                        rr = p2.tile([1, 512], fp32, tag="rr", bufs=2, name=f"rr{h}_{ic}")
                        nc.vector.reciprocal(rr, cps[64:65, :])
                        rb = p2.tile([64, 512], fp32, tag="rb", bufs=2, name=f"rb{h}_{ic}")
                        nc.gpsimd.partition_broadcast(rb, rr)
                        nc.vector.tensor_mul(
                            ctxT[h * 64:(h + 1) * 64, ic * 512:(ic + 1) * 512],
                            cps[0:64, :],
                            rb,
                        )

                # ---------------- phase 3: output projection ----------------
                for ib in range(NJB):
                    ob = p2.tile([128, HID], fp32, tag="ob", bufs=3, name=f"ob{ib}")
                    for oc in range(2):
                        ops = ps2.tile([128, 512], fp32, tag="out", bufs=2, name=f"ops{ib}_{oc}")
                        nc.tensor.matmul(
                            ops,
                            lhsT=ctxT[:, ib * 128:(ib + 1) * 128],
                            rhs=wo_sb[:, oc * 512:(oc + 1) * 512],
                            start=True,
                            stop=True,
                        )
                        nc.vector.tensor_copy(ob[:, oc * 512:(oc + 1) * 512], ops)
                    nc.sync.dma_start(
                        out=out[ib * 128:(ib + 1) * 128, :], in_=ob
                    )

    nc.compile()
    return nc


def kernel(x, Wq, bq, Wk, bk, Wv, bv, Wo, bo, cmw, mask, modality_info,
           _perf=None):
    from concourse.bass_utils import run_bass_kernel_spmd

    x = np.asarray(x, np.float32)
    Wq = np.asarray(Wq, np.float32)
    Wk = np.asarray(Wk, np.float32)
    Wv = np.asarray(Wv, np.float32)
    Wo = np.asarray(Wo, np.float32)
    bq_ = np.asarray(bq, np.float32)
    bk_ = np.asarray(bk, np.float32)
    bv_ = np.asarray(bv, np.float32)
    bo_ = np.asarray(bo, np.float32)
    cmw = np.asarray(cmw, np.float32)
    mask2 = np.asarray(mask)[0]
    mi = np.asarray(modality_info).astype(np.int64)[0]

    causal = bool(
        np.array_equal(mask2 != 0, np.tril(np.ones((S, S), bool)))
    )
    has_bq = bool(np.any(bq_))
    has_bk = bool(np.any(bk_))
    has_bv = bool(np.any(bv_))

    key = (causal, has_bq, has_bk, has_bv)
    if key not in _CACHE:
        _CACHE[key] = _build(*key)
    nc = _CACHE[key]

    scale = 1.0 / math.sqrt(D)
    # rank-3 factorization of the gathered cross-modal bias
    R = np.zeros((S, 3), np.float32)
    R[np.arange(S), mi] = 1.0
    U = R @ cmw
    uT4 = np.zeros((4, S), BF16)
    rT4 = np.zeros((4, S), BF16)
    uT4[0:3, :] = U.T.astype(BF16)
    rT4[0:3, :] = R.T.astype(BF16)
    xTb = np.ascontiguousarray(x[0].T).astype(BF16)

    in_maps = []
    for c in range(NCORES):
        sl = slice(c * DPC, (c + 1) * DPC)
        m = {
            "xT": xTb,
            # scores scale folded into the q-side weights (and bias)
            "wqT": np.ascontiguousarray(Wq[sl, :].T * scale).astype(BF16),
            "wkT": np.ascontiguousarray(Wk[sl, :].T).astype(BF16),
            "wvT": np.ascontiguousarray(Wv[sl, :].T).astype(BF16),
            "woT": np.ascontiguousarray(Wo[:, sl].T).astype(BF16),
            "uT": uT4,
            "rT": rT4,
        }
        if has_bq:
            m["bq"] = np.ascontiguousarray(bq_[sl, None] * scale)
        if has_bk:
            m["bk"] = np.ascontiguousarray(bk_[sl, None])
        if has_bv:
            m["bv"] = np.ascontiguousarray(bv_[None, sl])
        if not causal:
            m["maskT"] = np.ascontiguousarray(mask2.T != 0).astype(BF16)
        in_maps.append(m)

    res = run_bass_kernel_spmd(
        nc, in_maps, core_ids=list(range(NCORES)),
        trace=bool(_perf is not None),
    )
    outp = np.zeros((S, HID), np.float32)
    for r in res.results:
        outp += r["out"]
    outp += bo_[None, :]
    if _perf is not None:
        _perf["exec_time_ns"] = res.exec_time_ns
        _perf["trace"] = res.instructions_and_trace
    return outp.reshape(B, S, HID)


# revision 11
# speedup vs baseline: 1.1751x; 1.1751x over previous
# Trainium2 Bass kernel for nn_MultiHeadAttention_71674414235938
#
# MHA with a cross-modal additive bias gathered from a 3x3 table and a causal
# mask, B=1, S=2048, HID=1024, H=16 heads of D=64.
#
# Sharding: tensor-parallel over heads. 2 heads per core (dq slice of 128).
# Each core computes q/k/v projections for its heads, head-local attention,
# and a partial output ctx_c @ Wo[:, c*128:(c+1)*128].T which the host sums.
#
# Device-side layout choices:
#   * scores are computed TRANSPOSED: sT[j, i] = k[j]·q[i] (j on partitions),
#     so softmax-denominators and the attn@V contraction both run without any
#     on-chip transposes:  ctxT[d, i] = sum_j v'[j, d] * attnT[j, i]  with
#     lhsT = v' (natural layout) and rhs = attnT (as produced).
#   * the 3x3 cross-modal bias is rank-3:  bias = (onehot(m) @ cmw) @ onehot(m).T
#     so it is folded into the scores matmul by appending 3 rows (U.T to the
#     q side, R.T to the k side), K = 64+3 = 67.
#   * softmax runs without max-subtraction: scores are O(+-6) here, exp is
#     safely in fp32 range.
#   * a ones-column appended to v makes the PE accumulate the softmax
#     denominator into ctxT row 64; normalization happens on the way out of
#     PSUM (reciprocal + partition-broadcast DMA + multiply).
#   * causal structure: score blocks entirely above the diagonal are skipped;
#     diagonal staircase blocks are masked multiplicatively after exp.

import math

import numpy as np
import ml_dtypes

B, S, HID, H, D = 1, 2048, 1024, 16, 64
NCORES = 8
HPC = H // NCORES          # heads per core = 2
DPC = HPC * D              # head-dim columns per core = 128
KC = HID // 128            # contraction chunks = 8
NIC = S // 512             # 512-wide i-chunks = 4
NJB = S // 128             # 128-tall j-blocks = 16

BF16 = ml_dtypes.bfloat16

_CACHE = {}


def _build(causal: bool, has_bq: bool, has_bk: bool, has_bv: bool):
    import concourse.bass as bass
    import concourse.bacc as bacc
    import concourse.mybir as mybir
    import concourse.tile as tile

    fp32 = mybir.dt.float32
    bf16 = mybir.dt.bfloat16
    Exp = mybir.ActivationFunctionType.Exp

    nc = bacc.Bacc()

    xT = nc.declare_dram_parameter("xT", [HID, S], bf16, isOutput=False)
    wqT = nc.declare_dram_parameter("wqT", [HID, DPC], bf16, isOutput=False)
    wkT = nc.declare_dram_parameter("wkT", [HID, DPC], bf16, isOutput=False)
    wvT = nc.declare_dram_parameter("wvT", [HID, DPC], bf16, isOutput=False)
    woT = nc.declare_dram_parameter("woT", [DPC, HID], bf16, isOutput=False)
    uT = nc.declare_dram_parameter("uT", [4, S], bf16, isOutput=False)
    rT = nc.declare_dram_parameter("rT", [4, S], bf16, isOutput=False)
    if has_bq:
        bq = nc.declare_dram_parameter("bq", [DPC, 1], fp32, isOutput=False)
    if has_bk:
        bk = nc.declare_dram_parameter("bk", [DPC, 1], fp32, isOutput=False)
    if has_bv:
        bv = nc.declare_dram_parameter("bv", [1, DPC], fp32, isOutput=False)
    if not causal:
        maskT = nc.declare_dram_parameter("maskT", [S, S], bf16, isOutput=False)
    out = nc.declare_dram_parameter("out", [S, HID], fp32, isOutput=True)

    with tile.TileContext(nc) as tc:
        with tc.tile_pool(name="persist", bufs=1) as pp:
            # persistent SBUF tensors
            wo_sb = pp.tile([128, HID], bf16)
            nc.sync.dma_start(out=wo_sb, in_=woT[:, :])
            # qU / kR: per head, 67 live rows ([0:64] proj, [64:67] bias factors)
            qU = [pp.tile([67, S], bf16, name=f"qU{h}") for h in range(HPC)]
            kR = [pp.tile([67, S], bf16, name=f"kR{h}") for h in range(HPC)]
            for h in range(HPC):
                nc.sync.dma_start(out=qU[h][64:67, :], in_=uT[0:3, :])
                nc.sync.dma_start(out=kR[h][64:67, :], in_=rT[0:3, :])
            # v' tiles: [j%128, jb, 65] with ones in column 64
            vp = [pp.tile([128, NJB, 65], bf16, name=f"vp{h}") for h in range(HPC)]
            for h in range(HPC):
                nc.vector.memset(vp[h][:, :, 64:65], 1.0)
            # normalized transposed context, both heads: [dc, i]
            ctxT = pp.tile([128, S], bf16)
            # staircase causal mask for the diagonal 128-col strip: the strip
            # at[:, d:d+128] covers global cols i = jb*128 + f against rows
            # j = jb*128 + p, so keep (1.0) exactly where f >= p
            stair = None
            if causal:
                stair = pp.tile([128, 128], bf16)
                nc.vector.memset(stair, 1.0)
                nc.gpsimd.affine_select(
                    out=stair, in_=stair,
                    compare_op=mybir.AluOpType.is_ge,
                    fill=0.0,
                    base=0,
                    pattern=[[1, 128]],
                    channel_multiplier=-1,
                )
            if has_bq:
                bq_sb = pp.tile([DPC, 1], fp32)
                nc.sync.dma_start(out=bq_sb, in_=bq[:, :])
            if has_bk:
                bk_sb = pp.tile([DPC, 1], fp32)
                nc.sync.dma_start(out=bk_sb, in_=bk[:, :])
            if has_bv:
                bv_sb = pp.tile([128, DPC], fp32)
                bv_ap = bv[:, :]
                nc.gpsimd.dma_start(
                    out=bv_sb,
                    in_=bass.AP(tensor=bv_ap.tensor, offset=bv_ap.offset,
                                ap=[[0, 128], bv_ap.ap[1]]),
                )

            # ---------------- phase 1: QKV projections ----------------
            with (
                tc.tile_pool(name="ph1", bufs=1) as p1,
                tc.tile_pool(name="ps1", bufs=1, space="PSUM") as ps1,
            ):
                w_sbs = {}
                for nm, src in (("q", wqT), ("k", wkT), ("v", wvT)):
                    w_sb = w_sbs[nm] = p1.tile(
                        [128, KC, DPC], bf16, name=f"w{nm}_sb"
                    )
                    nc.sync.dma_start(
                        out=w_sb, in_=src[:, :].rearrange("(kc p) m -> p kc m", p=128)
                    )
                # x arrives in per-kc chunks so the first matmuls can start
                # while the rest of x is still in flight
                xT_sb = p1.tile([128, KC, S], bf16)
                xT_re = xT[:, :].rearrange("(kc p) n -> p kc n", p=128)
                for kc in range(KC):
                    nc.sync.dma_start(
                        out=xT_sb[:, kc, :], in_=xT_re[:, kc, :]
                    )

                # q, k transposed projections: out[dq, i], kc-outer so the
                # stationary operand is reused across the 4 i-chunks; q and k
                # accumulate simultaneously in 8 psum banks
                pss = {
                    nm: [
                        ps1.tile([128, 512], fp32, tag=f"p{off + n}",
                                 name=f"ps_{nm}{n}")
                        for n in range(NIC)
                    ]
                    for nm, off in (("q", 0), ("k", 4))
                }
                for kc in range(KC):
                    for nm in ("q", "k"):
                        for n in range(NIC):
                            nc.tensor.matmul(
                                pss[nm][n],
                                lhsT=w_sbs[nm][:, kc, :],
                                rhs=xT_sb[:, kc, n * 512:(n + 1) * 512],
                                start=(kc == 0),
                                stop=(kc == KC - 1),
                            )
                for nm, dsts, bias_sb in (
                    ("q", qU, bq_sb if has_bq else None),
                    ("k", kR, bk_sb if has_bk else None),
                ):
                    for n in range(NIC):
                        for h in range(HPC):
                            dst = dsts[h][0:64, n * 512:(n + 1) * 512]
                            src = pss[nm][n][h * 64:(h + 1) * 64, :]
                            if bias_sb is not None:
                                nc.vector.tensor_scalar_add(
                                    dst, src, bias_sb[h * 64:(h + 1) * 64, 0:1]
                                )
                            else:
                                nc.vector.tensor_copy(dst, src)

                # v natural projection: out[j, dv] so no transpose is needed;
                # cycles through the 8 freed q/k psum slots
                for jb in range(NJB):
                    psv = ps1.tile([128, DPC], fp32, tag=f"p{jb % 8}",
                                   name=f"psv{jb}")
                    for kc in range(KC):
                        nc.tensor.matmul(
                            psv,
                            lhsT=xT_sb[:, kc, jb * 128:(jb + 1) * 128],
                            rhs=w_sbs["v"][:, kc, :],
                            start=(kc == 0),
                            stop=(kc == KC - 1),
                        )
                    for h in range(HPC):
                        dst = vp[h][:, jb, 0:64]
                        src = psv[:, h * 64:(h + 1) * 64]
                        if has_bv:
                            nc.vector.tensor_add(
                                dst, src, bv_sb[:, h * 64:(h + 1) * 64]
                            )
                        else:
                            nc.vector.tensor_copy(dst, src)

            # ---------------- phase 2: attention per head ----------------
            with (
                tc.tile_pool(name="ph2", bufs=1) as p2,
                tc.tile_pool(name="ps2", bufs=1, space="PSUM") as ps2,
            ):
                for h in range(HPC):
                    at_tiles = []
                    for jb in range(NJB):
                        ics = (jb * 128) // 512 if causal else 0
                        w = S - ics * 512
                        at = p2.tile(
                            [128, w], bf16, tag=f"at{jb}", bufs=2, name=f"at{h}_{jb}"
                        )
                        for n in range(ics, NIC):
                            sc = ps2.tile([128, 512], fp32, tag="sc", bufs=4,
                                          name=f"sc{h}_{jb}_{n}")
                            nc.tensor.matmul(
                                sc,
                                lhsT=kR[h][:, jb * 128:(jb + 1) * 128],
                                rhs=qU[h][:, n * 512:(n + 1) * 512],
                                start=True,
                                stop=True,
                            )
                            nc.scalar.activation(
                                at[:, (n - ics) * 512:(n - ics + 1) * 512], sc, Exp
                            )
                        if causal:
                            d = (jb % 4) * 128
                            if d:
                                nc.gpsimd.memset(at[:, 0:d], 0.0)
                            nc.vector.tensor_mul(
                                at[:, d:d + 128], at[:, d:d + 128], stair
                            )
                        else:
                            mt = p2.tile([128, S], bf16, tag="mt", bufs=2, name=f"mt{h}_{jb}")
                            nc.sync.dma_start(
                                out=mt, in_=maskT[jb * 128:(jb + 1) * 128, :]
                            )
                            nc.vector.tensor_mul(at, at, mt)
                        at_tiles.append((at, ics))

                    for ic in range(NIC):
                        jmax = (ic + 1) * 4 if causal else NJB
                        cps = ps2.tile([65, 512], fp32, tag="ctx", bufs=2, name=f"cps{h}_{ic}")
                        for jb in range(jmax):
                            at, ics = at_tiles[jb]
                            nc.tensor.matmul(
                                cps,
                                lhsT=vp[h][:, jb, :],
                                rhs=at[:, (ic - ics) * 512:(ic - ics + 1) * 512],
                                start=(jb == 0),
                                stop=(jb == jmax - 1),
                            )
                        rr = p2.tile([1, 512], fp32, tag="rr", bufs=2, name=f"rr{h}_{ic}")
                        nc.vector.reciprocal(rr, cps[64:65, :])
                        rb = p2.tile([64, 512], fp32, tag="rb", bufs=2, name=f"rb{h}_{ic}")
                        nc.gpsimd.partition_broadcast(rb, rr)
                        nc.vector.tensor_mul(
                            ctxT[h * 64:(h + 1) * 64, ic * 512:(ic + 1) * 512],
                            cps[0:64, :],
                            rb,
                        )

                # ---------------- phase 3: output projection ----------------
                for ib in range(NJB):
                    ob = p2.tile([128, HID], fp32, tag="ob", bufs=3, name=f"ob{ib}")
                    for oc in range(2):
                        ops = ps2.tile([128, 512], fp32, tag="out", bufs=2, name=f"ops{ib}_{oc}")
                        nc.tensor.matmul(
                            ops,
                            lhsT=ctxT[:, ib * 128:(ib + 1) * 128],
                            rhs=wo_sb[:, oc * 512:(oc + 1) * 512],
                            start=True,
                            stop=True,
                        )
                        nc.vector.tensor_copy(ob[:, oc * 512:(oc + 1) * 512], ops)
                    nc.sync.dma_start(
                        out=out[ib * 128:(ib + 1) * 128, :], in_=ob
                    )

    nc.compile()
    return nc


def kernel(x, Wq, bq, Wk, bk, Wv, bv, Wo, bo, cmw, mask, modality_info,
           _perf=None):
    from concourse.bass_utils import run_bass_kernel_spmd

    x = np.asarray(x, np.float32)
    Wq = np.asarray(Wq, np.float32)
    Wk = np.asarray(Wk, np.float32)
    Wv = np.asarray(Wv, np.float32)
    Wo = np.asarray(Wo, np.float32)
    bq_ = np.asarray(bq, np.float32)
    bk_ = np.asarray(bk, np.float32)
    bv_ = np.asarray(bv, np.float32)
    bo_ = np.asarray(bo, np.float32)
    cmw = np.asarray(cmw, np.float32)
    mask2 = np.asarray(mask)[0]
    mi = np.asarray(modality_info).astype(np.int64)[0]

    causal = bool(
        np.array_equal(mask2 != 0, np.tril(np.ones((S, S), bool)))
    )
    has_bq = bool(np.any(bq_))
    has_bk = bool(np.any(bk_))
    has_bv = bool(np.any(bv_))

    key = (causal, has_bq, has_bk, has_bv)
    if key not in _CACHE:
        _CACHE[key] = _build(*key)
    nc = _CACHE[key]

    scale = 1.0 / math.sqrt(D)
    # rank-3 factorization of the gathered cross-modal bias
    R = np.zeros((S, 3), np.float32)
    R[np.arange(S), mi] = 1.0
    U = R @ cmw
    uT4 = np.zeros((4, S), BF16)
    rT4 = np.zeros((4, S), BF16)
    uT4[0:3, :] = U.T.astype(BF16)
    rT4[0:3, :] = R.T.astype(BF16)
    xTb = np.ascontiguousarray(x[0].T).astype(BF16)

    in_maps = []
    for c in range(NCORES):
        sl = slice(c * DPC, (c + 1) * DPC)
        m = {
            "xT": xTb,
            # scores scale folded into the q-side weights (and bias)
            "wqT": np.ascontiguousarray(Wq[sl, :].T * scale).astype(BF16),
            "wkT": np.ascontiguousarray(Wk[sl, :].T).astype(BF16),
            "wvT": np.ascontiguousarray(Wv[sl, :].T).astype(BF16),
            "woT": np.ascontiguousarray(Wo[:, sl].T).astype(BF16),
            "uT": uT4,
            "rT": rT4,
        }
        if has_bq:
            m["bq"] = np.ascontiguousarray(bq_[sl, None] * scale)
        if has_bk:
            m["bk"] = np.ascontiguousarray(bk_[sl, None])
        if has_bv:
            m["bv"] = np.ascontiguousarray(bv_[None, sl])
        if not causal:
            m["maskT"] = np.ascontiguousarray(mask2.T != 0).astype(BF16)
        in_maps.append(m)

    res = run_bass_kernel_spmd(
        nc, in_maps, core_ids=list(range(NCORES)),
        trace=bool(_perf is not None),
    )
    outp = np.zeros((S, HID), np.float32)
    for r in res.results:
        outp += r["out"]
    outp += bo_[None, :]
    if _perf is not None:
        _perf["exec_time_ns"] = res.exec_time_ns
        _perf["trace"] = res.instructions_and_trace
    return outp.reshape(B, S, HID)


# revision 14
# speedup vs baseline: 1.2002x; 1.0214x over previous
# Trainium2 Bass kernel for nn_MultiHeadAttention_71674414235938
#
# MHA with a cross-modal additive bias gathered from a 3x3 table and a causal
# mask, B=1, S=2048, HID=1024, H=16 heads of D=64.
#
# Sharding: tensor-parallel over heads. 2 heads per core (dq slice of 128).
# Each core computes q/k/v projections for its heads, head-local attention,
# and a partial output ctx_c @ Wo[:, c*128:(c+1)*128].T which the host sums.
#
# Device-side layout choices:
#   * scores are computed TRANSPOSED: sT[j, i] = k[j]·q[i] (j on partitions),
#     so softmax-denominators and the attn@V contraction both run without any
#     on-chip transposes:  ctxT[d, i] = sum_j v'[j, d] * attnT[j, i]  with
#     lhsT = v' (natural layout) and rhs = attnT (as produced).
#   * the 3x3 cross-modal bias is rank-3:  bias = (onehot(m) @ cmw) @ onehot(m).T
#     so it is folded into the scores matmul by appending 3 rows (U.T to the
#     q side, R.T to the k side), K = 64+3 = 67.
#   * softmax runs without max-subtraction: scores are O(+-6) here, exp is
#     safely in fp32 range.
#   * a ones-column appended to v makes the PE accumulate the softmax
#     denominator into ctxT row 64; normalization happens on the way out of
#     PSUM (reciprocal + partition-broadcast DMA + multiply).
#   * causal structure: score blocks entirely above the diagonal are skipped;
#     diagonal staircase blocks are masked multiplicatively after exp.

import math

import numpy as np
import ml_dtypes

B, S, HID, H, D = 1, 2048, 1024, 16, 64
NCORES = 8
HPC = H // NCORES          # heads per core = 2
DPC = HPC * D              # head-dim columns per core = 128
KC = HID // 128            # contraction chunks = 8
NIC = S // 512             # 512-wide i-chunks = 4
NJB = S // 128             # 128-tall j-blocks = 16

BF16 = ml_dtypes.bfloat16

_CACHE = {}


def _build(causal: bool, has_bq: bool, has_bk: bool, has_bv: bool):
    import concourse.bass as bass
    import concourse.bacc as bacc
    import concourse.mybir as mybir
    import concourse.tile as tile

    fp32 = mybir.dt.float32
    bf16 = mybir.dt.bfloat16
    Exp = mybir.ActivationFunctionType.Exp

    nc = bacc.Bacc()

    xT = nc.declare_dram_parameter("xT", [HID, S], bf16, isOutput=False)
    wqT = nc.declare_dram_parameter("wqT", [HID, DPC], bf16, isOutput=False)
    wkT = nc.declare_dram_parameter("wkT", [HID, DPC], bf16, isOutput=False)
    wvT = nc.declare_dram_parameter("wvT", [HID, DPC], bf16, isOutput=False)
    woT = nc.declare_dram_parameter("woT", [DPC, HID], bf16, isOutput=False)
    uT = nc.declare_dram_parameter("uT", [4, S], bf16, isOutput=False)
    rT = nc.declare_dram_parameter("rT", [4, S], bf16, isOutput=False)
    if has_bq:
        bq = nc.declare_dram_parameter("bq", [DPC, 1], fp32, isOutput=False)
    if has_bk:
        bk = nc.declare_dram_parameter("bk", [DPC, 1], fp32, isOutput=False)
    if has_bv:
        bv = nc.declare_dram_parameter("bv", [1, DPC], fp32, isOutput=False)
    if not causal:
        maskT = nc.declare_dram_parameter("maskT", [S, S], bf16, isOutput=False)
    out = nc.declare_dram_parameter("out", [S, HID], fp32, isOutput=True)

    with tile.TileContext(nc) as tc:
        with tc.tile_pool(name="persist", bufs=1) as pp:
            # persistent SBUF tensors
            wo_sb = pp.tile([128, HID], bf16)
            nc.sync.dma_start(out=wo_sb, in_=woT[:, :])
            # qU / kR: per head, 67 live rows ([0:64] proj, [64:67] bias factors)
            qU = [pp.tile([67, S], bf16, name=f"qU{h}") for h in range(HPC)]
            kR = [pp.tile([67, S], bf16, name=f"kR{h}") for h in range(HPC)]
            for h in range(HPC):
                nc.sync.dma_start(out=qU[h][64:67, :], in_=uT[0:3, :])
                nc.sync.dma_start(out=kR[h][64:67, :], in_=rT[0:3, :])
            # v' tiles: [j%128, jb, 65] with ones in column 64
            vp = [pp.tile([128, NJB, 65], bf16, name=f"vp{h}") for h in range(HPC)]
            for h in range(HPC):
                nc.vector.memset(vp[h][:, :, 64:65], 1.0)
            # normalized transposed context, both heads: [dc, i]
            ctxT = pp.tile([128, S], bf16)
            # staircase causal mask for the diagonal 128-col strip: the strip
            # at[:, d:d+128] covers global cols i = jb*128 + f against rows
            # j = jb*128 + p, so keep (1.0) exactly where f >= p
            stair = None
            if causal:
                stair = pp.tile([128, 128], bf16)
                nc.vector.memset(stair, 1.0)
                nc.gpsimd.affine_select(
                    out=stair, in_=stair,
                    compare_op=mybir.AluOpType.is_ge,
                    fill=0.0,
                    base=0,
                    pattern=[[1, 128]],
                    channel_multiplier=-1,
                )
            if has_bq:
                bq_sb = pp.tile([DPC, 1], fp32)
                nc.sync.dma_start(out=bq_sb, in_=bq[:, :])
            if has_bk:
                bk_sb = pp.tile([DPC, 1], fp32)
                nc.sync.dma_start(out=bk_sb, in_=bk[:, :])
            if has_bv:
                bv_sb = pp.tile([128, DPC], fp32)
                bv_ap = bv[:, :]
                nc.gpsimd.dma_start(
                    out=bv_sb,
                    in_=bass.AP(tensor=bv_ap.tensor, offset=bv_ap.offset,
                                ap=[[0, 128], bv_ap.ap[1]]),
                )

            # ---------------- phase 1: QKV projections ----------------
            with (
                tc.tile_pool(name="ph1", bufs=1) as p1,
                tc.tile_pool(name="ps1", bufs=1, space="PSUM") as ps1,
            ):
                w_sbs = {}
                for nm, src in (("q", wqT), ("k", wkT), ("v", wvT)):
                    w_sb = w_sbs[nm] = p1.tile(
                        [128, KC, DPC], bf16, name=f"w{nm}_sb"
                    )
                    nc.sync.dma_start(
                        out=w_sb, in_=src[:, :].rearrange("(kc p) m -> p kc m", p=128)
                    )
                # x arrives in per-kc chunks so the first matmuls can start
                # while the rest of x is still in flight
                xT_sb = p1.tile([128, KC, S], bf16)
                xT_re = xT[:, :].rearrange("(kc p) n -> p kc n", p=128)
                for kc in range(KC):
                    nc.sync.dma_start(
                        out=xT_sb[:, kc, :], in_=xT_re[:, kc, :]
                    )

                # q, k transposed projections: out[dq, i], kc-outer so the
                # stationary operand is reused across the 4 i-chunks; q and k
                # accumulate simultaneously in 8 psum banks
                pss = {
                    nm: [
                        ps1.tile([128, 512], fp32, tag=f"p{off + n}",
                                 name=f"ps_{nm}{n}")
                        for n in range(NIC)
                    ]
                    for nm, off in (("q", 0), ("k", 4))
                }
                for kc in range(KC):
                    for nm in ("q", "k"):
                        for n in range(NIC):
                            nc.tensor.matmul(
                                pss[nm][n],
                                lhsT=w_sbs[nm][:, kc, :],
                                rhs=xT_sb[:, kc, n * 512:(n + 1) * 512],
                                start=(kc == 0),
                                stop=(kc == KC - 1),
                            )
                for nm, dsts, bias_sb in (
                    ("q", qU, bq_sb if has_bq else None),
                    ("k", kR, bk_sb if has_bk else None),
                ):
                    for n in range(NIC):
                        for h in range(HPC):
                            dst = dsts[h][0:64, n * 512:(n + 1) * 512]
                            src = pss[nm][n][h * 64:(h + 1) * 64, :]
                            if bias_sb is not None:
                                nc.vector.tensor_scalar_add(
                                    dst, src, bias_sb[h * 64:(h + 1) * 64, 0:1]
                                )
                            else:
                                nc.vector.tensor_copy(dst, src)

                # v natural projection: out[j, dv] so no transpose is needed;
                # cycles through the 8 freed q/k psum slots
                for jb in range(NJB):
                    psv = ps1.tile([128, DPC], fp32, tag=f"p{jb % 8}",
                                   name=f"psv{jb}")
                    for kc in range(KC):
                        nc.tensor.matmul(
                            psv,
                            lhsT=xT_sb[:, kc, jb * 128:(jb + 1) * 128],
                            rhs=w_sbs["v"][:, kc, :],
                            start=(kc == 0),
                            stop=(kc == KC - 1),
                        )
                    for h in range(HPC):
                        dst = vp[h][:, jb, 0:64]
                        src = psv[:, h * 64:(h + 1) * 64]
                        if has_bv:
                            nc.vector.tensor_add(
                                dst, src, bv_sb[:, h * 64:(h + 1) * 64]
                            )
                        else:
                            nc.vector.tensor_copy(dst, src)

            # ---------------- phase 2: attention per head ----------------
            with (
                tc.tile_pool(name="ph2", bufs=1) as p2,
                tc.tile_pool(name="ps2", bufs=1, space="PSUM") as ps2,
            ):
                def emit_scores(h, jb, at_tiles):
                    ics = (jb * 128) // 512 if causal else 0
                    w = S - ics * 512
                    at = p2.tile(
                        [128, w], bf16, tag=f"at{jb}", bufs=2, name=f"at{h}_{jb}"
                    )
                    for n in range(ics, NIC):
                        sc = ps2.tile([128, 512], fp32, tag="sc", bufs=4,
                                      name=f"sc{h}_{jb}_{n}")
                        nc.tensor.matmul(
                            sc,
                            lhsT=kR[h][:, jb * 128:(jb + 1) * 128],
                            rhs=qU[h][:, n * 512:(n + 1) * 512],
                            start=True,
                            stop=True,
                        )
                        nc.scalar.activation(
                            at[:, (n - ics) * 512:(n - ics + 1) * 512], sc, Exp
                        )
                    if causal:
                        d = (jb % 4) * 128
                        if d:
                            nc.gpsimd.memset(at[:, 0:d], 0.0)
                        nc.vector.tensor_mul(
                            at[:, d:d + 128], at[:, d:d + 128], stair
                        )
                    else:
                        mt = p2.tile([128, S], bf16, tag="mt", bufs=2,
                                     name=f"mt{h}_{jb}")
                        nc.sync.dma_start(
                            out=mt, in_=maskT[jb * 128:(jb + 1) * 128, :]
                        )
                        nc.vector.tensor_mul(at, at, mt)
                    at_tiles.append((at, ics))

                def emit_ctx(h, ic, at_tiles):
                    jmax = (ic + 1) * 4 if causal else NJB
                    cps = ps2.tile([65, 512], fp32, tag="ctx", bufs=2,
                                   name=f"cps{h}_{ic}")
                    for jb in range(jmax):
                        at, ics = at_tiles[jb]
                        nc.tensor.matmul(
                            cps,
                            lhsT=vp[h][:, jb, :],
                            rhs=at[:, (ic - ics) * 512:(ic - ics + 1) * 512],
                            start=(jb == 0),
                            stop=(jb == jmax - 1),
                        )
                    # normalize: denom row -> sbuf -> broadcast -> 1/x -> mul
                    rr = p2.tile([1, 512], fp32, tag="rr", bufs=2,
                                 name=f"rr{h}_{ic}")
                    nc.vector.tensor_copy(rr, cps[64:65, :])
                    rb = p2.tile([64, 512], fp32, tag="rb", bufs=2,
                                 name=f"rb{h}_{ic}")
                    nc.gpsimd.partition_broadcast(rb, rr)
                    nc.vector.reciprocal_approx_fast(rb, rb)
                    nc.vector.tensor_mul(
                        ctxT[h * 64:(h + 1) * 64, ic * 512:(ic + 1) * 512],
                        cps[0:64, :],
                        rb,
                    )

                def emit_outproj(ib):
                    ob = p2.tile([128, HID], fp32, tag="ob", bufs=3,
                                 name=f"ob{ib}")
                    for oc in range(2):
                        ops = ps2.tile([128, 512], fp32, tag="out", bufs=2,
                                       name=f"ops{ib}_{oc}")
                        nc.tensor.matmul(
                            ops,
                            lhsT=ctxT[:, ib * 128:(ib + 1) * 128],
                            rhs=wo_sb[:, oc * 512:(oc + 1) * 512],
                            start=True,
                            stop=True,
                        )
                        # split the psum->sbuf copies between ACT and DVE
                        if oc == 0:
                            nc.scalar.activation(
                                ob[:, oc * 512:(oc + 1) * 512], ops,
                                mybir.ActivationFunctionType.Copy,
                            )
                        else:
                            nc.vector.tensor_copy(
                                ob[:, oc * 512:(oc + 1) * 512], ops
                            )
                    nc.sync.dma_start(
                        out=out[ib * 128:(ib + 1) * 128, :], in_=ob
                    )

                # interleaved emission: after the diagonal group of jb's for
                # i-chunk ic is in, its ctx chain can run; once both heads
                # have normalized an i-chunk, its out-projection rows follow
                for h in range(HPC):
                    at_tiles = []
                    if causal:
                        for ic in range(NIC):
                            for jb in range(4 * ic, 4 * (ic + 1)):
                                emit_scores(h, jb, at_tiles)
                            emit_ctx(h, ic, at_tiles)
                            if h == HPC - 1:
                                for ib in range(4 * ic, 4 * (ic + 1)):
                                    emit_outproj(ib)
                    else:
                        # dense mask: every ctx chain needs all j-blocks
                        for jb in range(NJB):
                            emit_scores(h, jb, at_tiles)
                        for ic in range(NIC):
                            emit_ctx(h, ic, at_tiles)
                            if h == HPC - 1:
                                for ib in range(4 * ic, 4 * (ic + 1)):
                                    emit_outproj(ib)

    nc.compile()
    return nc


def kernel(x, Wq, bq, Wk, bk, Wv, bv, Wo, bo, cmw, mask, modality_info,
           _perf=None):
    from concourse.bass_utils import run_bass_kernel_spmd

    x = np.asarray(x, np.float32)
    Wq = np.asarray(Wq, np.float32)
    Wk = np.asarray(Wk, np.float32)
    Wv = np.asarray(Wv, np.float32)
    Wo = np.asarray(Wo, np.float32)
    bq_ = np.asarray(bq, np.float32)
    bk_ = np.asarray(bk, np.float32)
    bv_ = np.asarray(bv, np.float32)
    bo_ = np.asarray(bo, np.float32)
    cmw = np.asarray(cmw, np.float32)
    mask2 = np.asarray(mask)[0]
    mi = np.asarray(modality_info).astype(np.int64)[0]

    causal = bool(
        np.array_equal(mask2 != 0, np.tril(np.ones((S, S), bool)))
    )
    has_bq = bool(np.any(bq_))
    has_bk = bool(np.any(bk_))
    has_bv = bool(np.any(bv_))

    key = (causal, has_bq, has_bk, has_bv)
    if key not in _CACHE:
        _CACHE[key] = _build(*key)
    nc = _CACHE[key]

    scale = 1.0 / math.sqrt(D)
    # rank-3 factorization of the gathered cross-modal bias
    R = np.zeros((S, 3), np.float32)
    R[np.arange(S), mi] = 1.0
    U = R @ cmw
    uT4 = np.zeros((4, S), BF16)
    rT4 = np.zeros((4, S), BF16)
    uT4[0:3, :] = U.T.astype(BF16)
    rT4[0:3, :] = R.T.astype(BF16)
    xTb = np.ascontiguousarray(x[0].T).astype(BF16)

    in_maps = []
    for c in range(NCORES):
        sl = slice(c * DPC, (c + 1) * DPC)
        m = {
            "xT": xTb,
            # scores scale folded into the q-side weights (and bias)
            "wqT": np.ascontiguousarray(Wq[sl, :].T * scale).astype(BF16),
            "wkT": np.ascontiguousarray(Wk[sl, :].T).astype(BF16),
            "wvT": np.ascontiguousarray(Wv[sl, :].T).astype(BF16),
            "woT": np.ascontiguousarray(Wo[:, sl].T).astype(BF16),
            "uT": uT4,
            "rT": rT4,
        }
        if has_bq:
            m["bq"] = np.ascontiguousarray(bq_[sl, None] * scale)
        if has_bk:
            m["bk"] = np.ascontiguousarray(bk_[sl, None])
        if has_bv:
            m["bv"] = np.ascontiguousarray(bv_[None, sl])
        if not causal:
            m["maskT"] = np.ascontiguousarray(mask2.T != 0).astype(BF16)
        in_maps.append(m)

    res = run_bass_kernel_spmd(
        nc, in_maps, core_ids=list(range(NCORES)),
        trace=bool(_perf is not None),
    )
    outp = np.zeros((S, HID), np.float32)
    for r in res.results:
        outp += r["out"]
    outp += bo_[None, :]
    if _perf is not None:
        _perf["exec_time_ns"] = res.exec_time_ns
        _perf["trace"] = res.instructions_and_trace
    return outp.reshape(B, S, HID)


# revision 16
# speedup vs baseline: 1.4639x; 1.2197x over previous
# Trainium2 Bass kernel for nn_MultiHeadAttention_71674414235938
#
# MHA with a cross-modal additive bias gathered from a 3x3 table and a causal
# mask, B=1, S=2048, HID=1024, H=16 heads of D=64.
#
# Sharding: tensor-parallel over heads. 2 heads per core (dq slice of 128).
# Each core computes q/k/v projections for its heads, head-local attention,
# and a partial output ctx_c @ Wo[:, c*128:(c+1)*128].T which the host sums.
#
# Device-side layout choices:
#   * scores are computed TRANSPOSED: sT[j, i] = k[j]·q[i] (j on partitions),
#     so softmax-denominators and the attn@V contraction both run without any
#     on-chip transposes:  ctxT[d, i] = sum_j v'[j, d] * attnT[j, i]  with
#     lhsT = v' (natural layout) and rhs = attnT (as produced).
#   * the 3x3 cross-modal bias is rank-3:  bias = (onehot(m) @ cmw) @ onehot(m).T
#     so it is folded into the scores matmul by appending 3 rows (U.T to the
#     q side, R.T to the k side), K = 64+3 = 67.
#   * softmax runs without max-subtraction: scores are O(+-6) here, exp is
#     safely in fp32 range.
#   * a ones-column appended to v makes the PE accumulate the softmax
#     denominator into ctxT row 64; normalization happens on the way out of
#     PSUM (reciprocal + partition-broadcast DMA + multiply).
#   * causal structure: score blocks entirely above the diagonal are skipped;
#     diagonal staircase blocks are masked multiplicatively after exp.

import math

import numpy as np
import ml_dtypes

B, S, HID, H, D = 1, 2048, 1024, 16, 64
NCORES = 8
HPC = H // NCORES          # heads per core = 2
DPC = HPC * D              # head-dim columns per core = 128
KC = HID // 128            # contraction chunks = 8
NIC = S // 512             # 512-wide i-chunks = 4
NJB = S // 128             # 128-tall j-blocks = 16

BF16 = ml_dtypes.bfloat16

_CACHE = {}


def _build(causal: bool, has_bq: bool, has_bk: bool, has_bv: bool):
    from contextlib import ExitStack

    import concourse.bass as bass
    import concourse.bacc as bacc
    import concourse.mybir as mybir
    import concourse.tile as tile

    fp32 = mybir.dt.float32
    bf16 = mybir.dt.bfloat16
    Exp = mybir.ActivationFunctionType.Exp
    Copy = mybir.ActivationFunctionType.Copy

    nc = bacc.Bacc()

    xT = nc.declare_dram_parameter("xT", [HID, S], bf16, isOutput=False)
    wqT = nc.declare_dram_parameter("wqT", [HID, DPC], bf16, isOutput=False)
    wkT = nc.declare_dram_parameter("wkT", [HID, DPC], bf16, isOutput=False)
    wvT = nc.declare_dram_parameter("wvT", [HID, DPC], bf16, isOutput=False)
    woT = nc.declare_dram_parameter("woT", [DPC, HID], bf16, isOutput=False)
    uT = nc.declare_dram_parameter("uT", [4, S], bf16, isOutput=False)
    rT = nc.declare_dram_parameter("rT", [4, S], bf16, isOutput=False)
    if has_bq:
        bq = nc.declare_dram_parameter("bq", [DPC, 1], fp32, isOutput=False)
    if has_bk:
        bk = nc.declare_dram_parameter("bk", [DPC, 1], fp32, isOutput=False)
    if has_bv:
        bv = nc.declare_dram_parameter("bv", [1, DPC], fp32, isOutput=False)
    if not causal:
        maskT = nc.declare_dram_parameter("maskT", [S, S], bf16, isOutput=False)
    out = nc.declare_dram_parameter("out", [S, HID], fp32, isOutput=True)

    with tile.TileContext(nc) as tc, ExitStack() as ctx:
        pp = ctx.enter_context(tc.tile_pool(name="persist", bufs=1))

        # -- input DMAs; critical path (wq/wk, x chunks) on the sync HWDGE
        #    queue, everything else on the gpsimd SWDGE queue
        w_sbs = {}
        for nm, src in (("q", wqT), ("k", wkT)):
            w_sb = w_sbs[nm] = pp.tile([128, KC, DPC], bf16, name=f"w{nm}_sb")
            nc.sync.dma_start(
                out=w_sb, in_=src[:, :].rearrange("(kc p) m -> p kc m", p=128)
            )
        xT_sb = pp.tile([128, KC, S], bf16)
        xT_re = xT[:, :].rearrange("(kc p) n -> p kc n", p=128)
        for kc in range(KC):
            nc.sync.dma_start(out=xT_sb[:, kc, :], in_=xT_re[:, kc, :])
        w_sbs["v"] = pp.tile([128, KC, DPC], bf16, name="wv_sb")
        nc.gpsimd.dma_start(
            out=w_sbs["v"],
            in_=wvT[:, :].rearrange("(kc p) m -> p kc m", p=128),
        )
        wo_sb = pp.tile([128, HID], bf16)
        nc.gpsimd.dma_start(out=wo_sb, in_=woT[:, :])

        # qU / kR: per head, 67 live rows ([0:64] proj, [64:67] bias factors)
        qU = [pp.tile([67, S], bf16, name=f"qU{h}") for h in range(HPC)]
        kR = [pp.tile([67, S], bf16, name=f"kR{h}") for h in range(HPC)]
        for h in range(HPC):
            nc.gpsimd.dma_start(out=qU[h][64:67, :], in_=uT[0:3, :])
            nc.gpsimd.dma_start(out=kR[h][64:67, :], in_=rT[0:3, :])
        # v': [j%128, jb, 65] with ones in column 64
        vp = [pp.tile([128, NJB, 65], bf16, name=f"vp{h}") for h in range(HPC)]
        for h in range(HPC):
            nc.vector.memset(vp[h][:, :, 64:65], 1.0)
        # normalized transposed context, both heads: [dc, i]
        ctxT = pp.tile([128, S], bf16)
        # staircase causal mask for a diagonal 128-col strip: keep iff f >= p
        stair = None
        if causal:
            stair = pp.tile([128, 128], bf16)
            nc.vector.memset(stair, 1.0)
            nc.gpsimd.affine_select(
                out=stair, in_=stair,
                compare_op=mybir.AluOpType.is_ge,
                fill=0.0, base=0,
                pattern=[[1, 128]],
                channel_multiplier=-1,
            )
            stair_b2 = bass.AP(
                tensor=stair.tensor, offset=stair.offset,
                ap=[stair.ap[0], [0, HPC], stair.ap[1]],
            )
        if has_bq:
            bq_sb = pp.tile([DPC, 1], fp32)
            nc.gpsimd.dma_start(out=bq_sb, in_=bq[:, :])
        if has_bk:
            bk_sb = pp.tile([DPC, 1], fp32)
            nc.gpsimd.dma_start(out=bk_sb, in_=bk[:, :])
        if has_bv:
            bv_sb = pp.tile([128, DPC], fp32)
            bv_ap = bv[:, :]
            nc.gpsimd.dma_start(
                out=bv_sb,
                in_=bass.AP(tensor=bv_ap.tensor, offset=bv_ap.offset,
                            ap=[[0, 128], bv_ap.ap[1]]),
            )

        # ---------------- q/k projections (8 psum banks) ----------------
        with tc.tile_pool(name="ps_qk", bufs=1, space="PSUM") as ps_qk:
            pss = {
                nm: [
                    ps_qk.tile([128, 512], fp32, tag=f"p{off + n}",
                               name=f"ps_{nm}{n}")
                    for n in range(NIC)
                ]
                for nm, off in (("q", 0), ("k", 4))
            }
            for kc in range(KC):
                for nm in ("q", "k"):
                    for n in range(NIC):
                        nc.tensor.matmul(
                            pss[nm][n],
                            lhsT=w_sbs[nm][:, kc, :],
                            rhs=xT_sb[:, kc, n * 512:(n + 1) * 512],
                            start=(kc == 0),
                            stop=(kc == KC - 1),
                        )
            for nm, dsts, bias_sb in (
                ("q", qU, bq_sb if has_bq else None),
                ("k", kR, bk_sb if has_bk else None),
            ):
                for n in range(NIC):
                    for h in range(HPC):
                        dst = dsts[h][0:64, n * 512:(n + 1) * 512]
                        src = pss[nm][n][h * 64:(h + 1) * 64, :]
                        if bias_sb is not None:
                            nc.vector.tensor_scalar_add(
                                dst, src, bias_sb[h * 64:(h + 1) * 64, 0:1]
                            )
                        else:
                            nc.vector.tensor_copy(dst, src)

        # ---------------- attention (+ overlapped v projection) ----------
        p2 = ctx.enter_context(tc.tile_pool(name="ph2", bufs=1))
        ps_sc = ctx.enter_context(
            tc.tile_pool(name="ps_sc", bufs=1, space="PSUM"))

        at_tiles = []  # (at2 [128, HPC, w], ics) per jb

        def emit_scores(jb):
            ics = (jb * 128) // 512 if causal else 0
            w = S - ics * 512
            at = p2.tile([128, HPC, w], bf16, tag=f"at{jb}", name=f"at{jb}")
            for n in range(ics, NIC):
                sc = ps_sc.tile([128, HPC * 512], fp32, tag="sc", bufs=2,
                                name=f"sc{jb}_{n}")
                for h in range(HPC):
                    nc.tensor.matmul(
                        sc[:, h * 512:(h + 1) * 512],
                        lhsT=kR[h][:, jb * 128:(jb + 1) * 128],
                        rhs=qU[h][:, n * 512:(n + 1) * 512],
                        start=True,
                        stop=True,
                    )
                nc.scalar.activation(
                    at[:, :, (n - ics) * 512:(n - ics + 1) * 512],
                    sc[:, :].rearrange("p (h n) -> p h n", h=HPC),
                    Exp,
                )
            if causal:
                d = (jb % 4) * 128
                if d:
                    nc.gpsimd.memset(at[:, :, 0:d], 0.0)
                nc.vector.tensor_mul(
                    at[:, :, d:d + 128], at[:, :, d:d + 128], stair_b2
                )
            else:
                mt = p2.tile([128, S], bf16, tag="mt", bufs=2, name=f"mt{jb}")
                nc.sync.dma_start(out=mt, in_=maskT[jb * 128:(jb + 1) * 128, :])
                mt_b2 = bass.AP(
                    tensor=mt.tensor, offset=mt.offset,
                    ap=[mt.ap[0], [0, HPC], mt.ap[1]],
                )
                nc.vector.tensor_mul(at, at, mt_b2)
            at_tiles.append((at, ics))

        def emit_ctx(h, ic, ps_cx):
            jmax = (ic + 1) * 4 if causal else NJB
            cps = ps_cx.tile([65, 512], fp32, tag="ctx", bufs=2,
                             name=f"cps{h}_{ic}")
            for jb in range(jmax):
                at, ics = at_tiles[jb]
                nc.tensor.matmul(
                    cps,
                    lhsT=vp[h][:, jb, :],
                    rhs=at[:, h, (ic - ics) * 512:(ic - ics + 1) * 512],
                    start=(jb == 0),
                    stop=(jb == jmax - 1),
                )
            # normalize: denom row -> sbuf -> broadcast -> 1/x -> mul
            rr = p2.tile([1, 512], fp32, tag="rr", bufs=2, name=f"rr{h}_{ic}")
            nc.vector.tensor_copy(rr, cps[64:65, :])
            rb = p2.tile([64, 512], fp32, tag="rb", bufs=2, name=f"rb{h}_{ic}")
            nc.gpsimd.partition_broadcast(rb, rr)
            nc.vector.reciprocal_approx_fast(rb, rb)
            nc.vector.tensor_mul(
                ctxT[h * 64:(h + 1) * 64, ic * 512:(ic + 1) * 512],
                cps[0:64, :],
                rb,
            )

        def emit_outproj(ib, ps_cx):
            ob = p2.tile([128, HID], fp32, tag="ob", bufs=3, name=f"ob{ib}")
            for oc in range(2):
                ops = ps_cx.tile([128, 512], fp32, tag="out", bufs=2,
                                 name=f"ops{ib}_{oc}")
                nc.tensor.matmul(
                    ops,
                    lhsT=ctxT[:, ib * 128:(ib + 1) * 128],
                    rhs=wo_sb[:, oc * 512:(oc + 1) * 512],
                    start=True,
                    stop=True,
                )
                if oc == 0:
                    nc.scalar.activation(ob[:, oc * 512:(oc + 1) * 512],
                                         ops, Copy)
                else:
                    nc.vector.tensor_copy(ob[:, oc * 512:(oc + 1) * 512], ops)
            nc.sync.dma_start(out=out[ib * 128:(ib + 1) * 128, :], in_=ob)

        # v projection (4 psum banks) overlapped with the first half of the
        # scores sweep; remaining psum opens for ctx/out once v is done
        with tc.tile_pool(name="ps_v", bufs=1, space="PSUM") as ps_v:
            for jb in range(8):
                emit_scores(jb)
                for vjb in (2 * jb, 2 * jb + 1):
                    psv = ps_v.tile([128, DPC], fp32, tag=f"v{vjb % 4}",
                                    name=f"psv{vjb}")
                    for kc in range(KC):
                        nc.tensor.matmul(
                            psv,
                            lhsT=xT_sb[:, kc, vjb * 128:(vjb + 1) * 128],
                            rhs=w_sbs["v"][:, kc, :],
                            start=(kc == 0),
                            stop=(kc == KC - 1),
                        )
                    for h in range(HPC):
                        dst = vp[h][:, vjb, 0:64]
                        src = psv[:, h * 64:(h + 1) * 64]
                        if has_bv:
                            nc.vector.tensor_add(
                                dst, src, bv_sb[:, h * 64:(h + 1) * 64]
                            )
                        else:
                            nc.vector.tensor_copy(dst, src)

        ps_cx = ctx.enter_context(
            tc.tile_pool(name="ps_cx", bufs=1, space="PSUM"))
        if causal:
            for ic in range(2):
                for h in range(HPC):
                    emit_ctx(h, ic, ps_cx)
                for ib in range(4 * ic, 4 * (ic + 1)):
                    emit_outproj(ib, ps_cx)
            for ic in range(2, NIC):
                for jb in range(4 * ic, 4 * (ic + 1)):
                    emit_scores(jb)
                for h in range(HPC):
                    emit_ctx(h, ic, ps_cx)
                for ib in range(4 * ic, 4 * (ic + 1)):
                    emit_outproj(ib, ps_cx)
        else:
            for jb in range(8, NJB):
                emit_scores(jb)
            for ic in range(NIC):
                for h in range(HPC):
                    emit_ctx(h, ic, ps_cx)
                for ib in range(4 * ic, 4 * (ic + 1)):
                    emit_outproj(ib, ps_cx)

    nc.compile()
    return nc



def kernel(x, Wq, bq, Wk, bk, Wv, bv, Wo, bo, cmw, mask, modality_info,
           _perf=None):
    from concourse.bass_utils import run_bass_kernel_spmd

    x = np.asarray(x, np.float32)
    Wq = np.asarray(Wq, np.float32)
    Wk = np.asarray(Wk, np.float32)
    Wv = np.asarray(Wv, np.float32)
    Wo = np.asarray(Wo, np.float32)
    bq_ = np.asarray(bq, np.float32)
    bk_ = np.asarray(bk, np.float32)
    bv_ = np.asarray(bv, np.float32)
    bo_ = np.asarray(bo, np.float32)
    cmw = np.asarray(cmw, np.float32)
    mask2 = np.asarray(mask)[0]
    mi = np.asarray(modality_info).astype(np.int64)[0]

    causal = bool(
        np.array_equal(mask2 != 0, np.tril(np.ones((S, S), bool)))
    )
    has_bq = bool(np.any(bq_))
    has_bk = bool(np.any(bk_))
    has_bv = bool(np.any(bv_))

    key = (causal, has_bq, has_bk, has_bv)
    if key not in _CACHE:
        _CACHE[key] = _build(*key)
    nc = _CACHE[key]

    scale = 1.0 / math.sqrt(D)
    # rank-3 factorization of the gathered cross-modal bias
    R = np.zeros((S, 3), np.float32)
    R[np.arange(S), mi] = 1.0
    U = R @ cmw
    uT4 = np.zeros((4, S), BF16)
    rT4 = np.zeros((4, S), BF16)
    uT4[0:3, :] = U.T.astype(BF16)
    rT4[0:3, :] = R.T.astype(BF16)
    xTb = np.ascontiguousarray(x[0].T).astype(BF16)

    in_maps = []
    for c in range(NCORES):
        sl = slice(c * DPC, (c + 1) * DPC)
        m = {
            "xT": xTb,
            # scores scale folded into the q-side weights (and bias)
            "wqT": np.ascontiguousarray(Wq[sl, :].T * scale).astype(BF16),
            "wkT": np.ascontiguousarray(Wk[sl, :].T).astype(BF16),
            "wvT": np.ascontiguousarray(Wv[sl, :].T).astype(BF16),
            "woT": np.ascontiguousarray(Wo[:, sl].T).astype(BF16),
            "uT": uT4,
            "rT": rT4,
        }
        if has_bq:
            m["bq"] = np.ascontiguousarray(bq_[sl, None] * scale)
        if has_bk:
            m["bk"] = np.ascontiguousarray(bk_[sl, None])
        if has_bv:
            m["bv"] = np.ascontiguousarray(bv_[None, sl])
        if not causal:
            m["maskT"] = np.ascontiguousarray(mask2.T != 0).astype(BF16)
        in_maps.append(m)

    res = run_bass_kernel_spmd(
        nc, in_maps, core_ids=list(range(NCORES)),
        trace=bool(_perf is not None),
    )
    outp = np.zeros((S, HID), np.float32)
    for r in res.results:
        outp += r["out"]
    outp += bo_[None, :]
    if _perf is not None:
        _perf["exec_time_ns"] = res.exec_time_ns
        _perf["trace"] = res.instructions_and_trace
    return outp.reshape(B, S, HID)


# revision 17
# speedup vs baseline: 1.5902x; 1.0862x over previous
# Trainium2 Bass kernel for nn_MultiHeadAttention_71674414235938
#
# MHA with a cross-modal additive bias gathered from a 3x3 table and a causal
# mask, B=1, S=2048, HID=1024, H=16 heads of D=64.
#
# Sharding: tensor-parallel over heads. 2 heads per core (dq slice of 128).
# Each core computes q/k/v projections for its heads, head-local attention,
# and a partial output ctx_c @ Wo[:, c*128:(c+1)*128].T which the host sums.
#
# Device-side layout choices:
#   * scores are computed TRANSPOSED: sT[j, i] = k[j]·q[i] (j on partitions),
#     so softmax-denominators and the attn@V contraction both run without any
#     on-chip transposes:  ctxT[d, i] = sum_j v'[j, d] * attnT[j, i]  with
#     lhsT = v' (natural layout) and rhs = attnT (as produced).
#   * the 3x3 cross-modal bias is rank-3:  bias = (onehot(m) @ cmw) @ onehot(m).T
#     so it is folded into the scores matmul by appending 3 rows (U.T to the
#     q side, R.T to the k side), K = 64+3 = 67.
#   * softmax runs without max-subtraction: scores are O(+-6) here, exp is
#     safely in fp32 range.
#   * a ones-column appended to v makes the PE accumulate the softmax
#     denominator into ctxT row 64; normalization happens on the way out of
#     PSUM (reciprocal + partition-broadcast DMA + multiply).
#   * causal structure: score blocks entirely above the diagonal are skipped;
#     diagonal staircase blocks are masked multiplicatively after exp.

import math

import numpy as np
import ml_dtypes

B, S, HID, H, D = 1, 2048, 1024, 16, 64
NCORES = 8
HPC = H // NCORES          # heads per core = 2
DPC = HPC * D              # head-dim columns per core = 128
KC = HID // 128            # contraction chunks = 8
NIC = S // 512             # 512-wide i-chunks = 4
NJB = S // 128             # 128-tall j-blocks = 16

BF16 = ml_dtypes.bfloat16

_CACHE = {}


def _build(causal: bool, has_bq: bool, has_bk: bool, has_bv: bool):
    from contextlib import ExitStack

    import concourse.bass as bass
    import concourse.bacc as bacc
    import concourse.mybir as mybir
    import concourse.tile as tile

    fp32 = mybir.dt.float32
    bf16 = mybir.dt.bfloat16
    Exp = mybir.ActivationFunctionType.Exp
    Copy = mybir.ActivationFunctionType.Copy

    nc = bacc.Bacc()

    xT = nc.declare_dram_parameter("xT", [HID, S], bf16, isOutput=False)
    wqT = nc.declare_dram_parameter("wqT", [HID, DPC], bf16, isOutput=False)
    wkT = nc.declare_dram_parameter("wkT", [HID, DPC], bf16, isOutput=False)
    wvT = nc.declare_dram_parameter("wvT", [HID, DPC], bf16, isOutput=False)
    woT = nc.declare_dram_parameter("woT", [DPC, HID], bf16, isOutput=False)
    uT = nc.declare_dram_parameter("uT", [4, S], bf16, isOutput=False)
    rT = nc.declare_dram_parameter("rT", [4, S], bf16, isOutput=False)
    if has_bq:
        bq = nc.declare_dram_parameter("bq", [DPC, 1], fp32, isOutput=False)
    if has_bk:
        bk = nc.declare_dram_parameter("bk", [DPC, 1], fp32, isOutput=False)
    if has_bv:
        bv = nc.declare_dram_parameter("bv", [1, DPC], fp32, isOutput=False)
    if not causal:
        maskT = nc.declare_dram_parameter("maskT", [S, S], bf16, isOutput=False)
    out = nc.declare_dram_parameter("out", [S, HID], fp32, isOutput=True)

    with tile.TileContext(nc) as tc, ExitStack() as ctx:
        pp = ctx.enter_context(tc.tile_pool(name="persist", bufs=1))

        # -- input DMAs; critical path (wq/wk, x chunks) on the sync HWDGE
        #    queue, everything else on the gpsimd SWDGE queue
        w_sbs = {}
        for nm, src in (("q", wqT), ("k", wkT)):
            w_sb = w_sbs[nm] = pp.tile([128, KC, DPC], bf16, name=f"w{nm}_sb")
            nc.sync.dma_start(
                out=w_sb, in_=src[:, :].rearrange("(kc p) m -> p kc m", p=128)
            )
        xT_re = xT[:, :].rearrange("(kc p) n -> p kc n", p=128)
        xT_sb = []
        for kc in range(KC):
            xk = pp.tile([128, S], bf16, name=f"xk{kc}")
            nc.sync.dma_start(out=xk, in_=xT_re[:, kc, :])
            xT_sb.append(xk)
        w_sbs["v"] = pp.tile([128, KC, DPC], bf16, name="wv_sb")
        nc.gpsimd.dma_start(
            out=w_sbs["v"],
            in_=wvT[:, :].rearrange("(kc p) m -> p kc m", p=128),
        )
        wo_sb = pp.tile([128, HID], bf16)
        nc.gpsimd.dma_start(out=wo_sb, in_=woT[:, :])

        # qU / kR: per head, 67 live rows ([0:64] proj, [64:67] bias factors)
        qU = [pp.tile([67, S], bf16, name=f"qU{h}") for h in range(HPC)]
        kR = [pp.tile([67, S], bf16, name=f"kR{h}") for h in range(HPC)]
        for h in range(HPC):
            nc.gpsimd.dma_start(out=qU[h][64:67, :], in_=uT[0:3, :])
            nc.gpsimd.dma_start(out=kR[h][64:67, :], in_=rT[0:3, :])
        # v': per (head, j-block) [128, 65] with ones in column 64
        vp = [[pp.tile([128, 65], bf16, name=f"vp{h}_{jb}") for jb in range(NJB)]
              for h in range(HPC)]
        for h in range(HPC):
            for jb in range(NJB):
                nc.gpsimd.memset(vp[h][jb][:, 64:65], 1.0)
        # normalized transposed context, both heads, one tile per i-chunk
        ctxT = [pp.tile([128, 512], bf16, name=f"ctxT{ic}") for ic in range(NIC)]
        # staircase causal mask for a diagonal 128-col strip: keep iff f >= p
        stair = None
        if causal:
            stair = pp.tile([128, 128], bf16)
            nc.vector.memset(stair, 1.0)
            nc.gpsimd.affine_select(
                out=stair, in_=stair,
                compare_op=mybir.AluOpType.is_ge,
                fill=0.0, base=0,
                pattern=[[1, 128]],
                channel_multiplier=-1,
            )
            stair_b2 = bass.AP(
                tensor=stair.tensor, offset=stair.offset,
                ap=[stair.ap[0], [0, HPC], stair.ap[1]],
            )
        if has_bq:
            bq_sb = pp.tile([DPC, 1], fp32)
            nc.gpsimd.dma_start(out=bq_sb, in_=bq[:, :])
        if has_bk:
            bk_sb = pp.tile([DPC, 1], fp32)
            nc.gpsimd.dma_start(out=bk_sb, in_=bk[:, :])
        if has_bv:
            bv_sb = pp.tile([128, DPC], fp32)
            bv_ap = bv[:, :]
            nc.gpsimd.dma_start(
                out=bv_sb,
                in_=bass.AP(tensor=bv_ap.tensor, offset=bv_ap.offset,
                            ap=[[0, 128], bv_ap.ap[1]]),
            )

        # ---------------- q/k projections (8 psum banks) ----------------
        with tc.tile_pool(name="ps_qk", bufs=1, space="PSUM") as ps_qk:
            pss = {
                nm: [
                    ps_qk.tile([128, 512], fp32, tag=f"p{off + n}",
                               name=f"ps_{nm}{n}")
                    for n in range(NIC)
                ]
                for nm, off in (("q", 0), ("k", 4))
            }
            for kc in range(KC):
                for nm in ("q", "k"):
                    for n in range(NIC):
                        nc.tensor.matmul(
                            pss[nm][n],
                            lhsT=w_sbs[nm][:, kc, :],
                            rhs=xT_sb[kc][:, n * 512:(n + 1) * 512],
                            start=(kc == 0),
                            stop=(kc == KC - 1),
                        )
            for nm, dsts, bias_sb in (
                ("q", qU, bq_sb if has_bq else None),
                ("k", kR, bk_sb if has_bk else None),
            ):
                for n in range(NIC):
                    for h in range(HPC):
                        dst = dsts[h][0:64, n * 512:(n + 1) * 512]
                        sr = pss[nm][n][h * 64:(h + 1) * 64, :]
                        if bias_sb is not None:
                            nc.vector.tensor_scalar_add(
                                dst, sr, bias_sb[h * 64:(h + 1) * 64, 0:1]
                            )
                        elif h == 0:
                            nc.vector.tensor_copy(dst, sr)
                        else:
                            nc.scalar.activation(dst, sr, Copy)

        # ---------------- attention (+ overlapped v projection) ----------
        p2 = ctx.enter_context(tc.tile_pool(name="ph2", bufs=1))
        ps_sc = ctx.enter_context(
            tc.tile_pool(name="ps_sc", bufs=1, space="PSUM"))

        at_tiles = []  # (at2 [128, HPC, w], ics) per jb

        def emit_scores(jb):
            ics = (jb * 128) // 512 if causal else 0
            w = S - ics * 512
            at = p2.tile([128, HPC, w], bf16, tag=f"at{jb}", name=f"at{jb}")
            for n in range(ics, NIC):
                sc = ps_sc.tile([128, HPC * 512], fp32, tag="sc", bufs=2,
                                name=f"sc{jb}_{n}")
                for h in range(HPC):
                    nc.tensor.matmul(
                        sc[:, h * 512:(h + 1) * 512],
                        lhsT=kR[h][:, jb * 128:(jb + 1) * 128],
                        rhs=qU[h][:, n * 512:(n + 1) * 512],
                        start=True,
                        stop=True,
                    )
                nc.scalar.activation(
                    at[:, :, (n - ics) * 512:(n - ics + 1) * 512],
                    sc[:, :].rearrange("p (h n) -> p h n", h=HPC),
                    Exp,
                )
            if causal:
                d = (jb % 4) * 128
                if d:
                    nc.gpsimd.memset(at[:, :, 0:d], 0.0)
                nc.vector.tensor_mul(
                    at[:, :, d:d + 128], at[:, :, d:d + 128], stair_b2
                )
            else:
                mt = p2.tile([128, S], bf16, tag="mt", bufs=2, name=f"mt{jb}")
                nc.sync.dma_start(out=mt, in_=maskT[jb * 128:(jb + 1) * 128, :])
                mt_b2 = bass.AP(
                    tensor=mt.tensor, offset=mt.offset,
                    ap=[mt.ap[0], [0, HPC], mt.ap[1]],
                )
                nc.vector.tensor_mul(at, at, mt_b2)
            at_tiles.append((at, ics))

        def emit_ctx(h, ic, ps_cx):
            jmax = (ic + 1) * 4 if causal else NJB
            cps = ps_cx.tile([65, 512], fp32, tag="ctx", bufs=2,
                             name=f"cps{h}_{ic}")
            for jb in range(jmax):
                at, ics = at_tiles[jb]
                nc.tensor.matmul(
                    cps,
                    lhsT=vp[h][jb],
                    rhs=at[:, h, (ic - ics) * 512:(ic - ics + 1) * 512],
                    start=(jb == 0),
                    stop=(jb == jmax - 1),
                )
            # normalize: denom row -> sbuf -> broadcast -> 1/x -> mul
            rr = p2.tile([1, 512], fp32, tag="rr", bufs=2, name=f"rr{h}_{ic}")
            nc.vector.tensor_copy(rr, cps[64:65, :])
            rb = p2.tile([64, 512], fp32, tag="rb", bufs=2, name=f"rb{h}_{ic}")
            nc.gpsimd.partition_broadcast(rb, rr)
            nc.vector.reciprocal_approx_fast(rb, rb)
            nc.vector.tensor_mul(
                ctxT[ic][h * 64:(h + 1) * 64, :],
                cps[0:64, :],
                rb,
            )

        def emit_outproj(ib, ps_cx):
            ob = p2.tile([128, HID], fp32, tag="ob", bufs=3, name=f"ob{ib}")
            for oc in range(2):
                ops = ps_cx.tile([128, 512], fp32, tag="out", bufs=2,
                                 name=f"ops{ib}_{oc}")
                nc.tensor.matmul(
                    ops,
                    lhsT=ctxT[ib // 4][:, (ib % 4) * 128:(ib % 4 + 1) * 128],
                    rhs=wo_sb[:, oc * 512:(oc + 1) * 512],
                    start=True,
                    stop=True,
                )
                if oc == 0:
                    nc.scalar.activation(ob[:, oc * 512:(oc + 1) * 512],
                                         ops, Copy)
                else:
                    nc.vector.tensor_copy(ob[:, oc * 512:(oc + 1) * 512], ops)
            nc.sync.dma_start(out=out[ib * 128:(ib + 1) * 128, :], in_=ob)

        # v projection (4 psum banks) overlapped with the first half of the
        # scores sweep; remaining psum opens for ctx/out once v is done
        with tc.tile_pool(name="ps_v", bufs=1, space="PSUM") as ps_v:
            for jb in range(8):
                emit_scores(jb)
                for vjb in (2 * jb, 2 * jb + 1):
                    psv = ps_v.tile([128, DPC], fp32, tag=f"v{vjb % 4}",
                                    name=f"psv{vjb}")
                    for kc in range(KC):
                        nc.tensor.matmul(
                            psv,
                            lhsT=xT_sb[kc][:, vjb * 128:(vjb + 1) * 128],
                            rhs=w_sbs["v"][:, kc, :],
                            start=(kc == 0),
                            stop=(kc == KC - 1),
                        )
                    for h in range(HPC):
                        dst = vp[h][vjb][:, 0:64]
                        src = psv[:, h * 64:(h + 1) * 64]
                        if has_bv:
                            nc.vector.tensor_add(
                                dst, src, bv_sb[:, h * 64:(h + 1) * 64]
                            )
                        else:
                            nc.vector.tensor_copy(dst, src)

        ps_cx = ctx.enter_context(
            tc.tile_pool(name="ps_cx", bufs=1, space="PSUM"))
        if causal:
            for ic in range(2):
                for h in range(HPC):
                    emit_ctx(h, ic, ps_cx)
                for ib in range(4 * ic, 4 * (ic + 1)):
                    emit_outproj(ib, ps_cx)
            for ic in range(2, NIC):
                for jb in range(4 * ic, 4 * (ic + 1)):
                    emit_scores(jb)
                for h in range(HPC):
                    emit_ctx(h, ic, ps_cx)
                for ib in range(4 * ic, 4 * (ic + 1)):
                    emit_outproj(ib, ps_cx)
        else:
            for jb in range(8, NJB):
                emit_scores(jb)
            for ic in range(NIC):
                for h in range(HPC):
                    emit_ctx(h, ic, ps_cx)
                for ib in range(4 * ic, 4 * (ic + 1)):
                    emit_outproj(ib, ps_cx)

    nc.compile()
    return nc



def kernel(x, Wq, bq, Wk, bk, Wv, bv, Wo, bo, cmw, mask, modality_info,
           _perf=None):
    from concourse.bass_utils import run_bass_kernel_spmd

    x = np.asarray(x, np.float32)
    Wq = np.asarray(Wq, np.float32)
    Wk = np.asarray(Wk, np.float32)
    Wv = np.asarray(Wv, np.float32)
    Wo = np.asarray(Wo, np.float32)
    bq_ = np.asarray(bq, np.float32)
    bk_ = np.asarray(bk, np.float32)
    bv_ = np.asarray(bv, np.float32)
    bo_ = np.asarray(bo, np.float32)
    cmw = np.asarray(cmw, np.float32)
    mask2 = np.asarray(mask)[0]
    mi = np.asarray(modality_info).astype(np.int64)[0]

    causal = bool(
        np.array_equal(mask2 != 0, np.tril(np.ones((S, S), bool)))
    )
    has_bq = bool(np.any(bq_))
    has_bk = bool(np.any(bk_))
    has_bv = bool(np.any(bv_))

    key = (causal, has_bq, has_bk, has_bv)
    if key not in _CACHE:
        _CACHE[key] = _build(*key)
    nc = _CACHE[key]

    scale = 1.0 / math.sqrt(D)
    # rank-3 factorization of the gathered cross-modal bias
    R = np.zeros((S, 3), np.float32)
    R[np.arange(S), mi] = 1.0
    U = R @ cmw
    uT4 = np.zeros((4, S), BF16)
    rT4 = np.zeros((4, S), BF16)
    uT4[0:3, :] = U.T.astype(BF16)
    rT4[0:3, :] = R.T.astype(BF16)
    xTb = np.ascontiguousarray(x[0].T).astype(BF16)

    in_maps = []
    for c in range(NCORES):
        sl = slice(c * DPC, (c + 1) * DPC)
        m = {
            "xT": xTb,
            # scores scale folded into the q-side weights (and bias)
            "wqT": np.ascontiguousarray(Wq[sl, :].T * scale).astype(BF16),
            "wkT": np.ascontiguousarray(Wk[sl, :].T).astype(BF16),
            "wvT": np.ascontiguousarray(Wv[sl, :].T).astype(BF16),
            "woT": np.ascontiguousarray(Wo[:, sl].T).astype(BF16),
            "uT": uT4,
            "rT": rT4,
        }
        if has_bq:
            m["bq"] = np.ascontiguousarray(bq_[sl, None] * scale)
        if has_bk:
            m["bk"] = np.ascontiguousarray(bk_[sl, None])
        if has_bv:
            m["bv"] = np.ascontiguousarray(bv_[None, sl])
        if not causal:
            m["maskT"] = np.ascontiguousarray(mask2.T != 0).astype(BF16)
        in_maps.append(m)

    res = run_bass_kernel_spmd(
        nc, in_maps, core_ids=list(range(NCORES)),
        trace=bool(_perf is not None),
    )
    outp = np.zeros((S, HID), np.float32)
    for r in res.results:
        outp += r["out"]
    outp += bo_[None, :]
    if _perf is not None:
        _perf["exec_time_ns"] = res.exec_time_ns
        _perf["trace"] = res.instructions_and_trace
    return outp.reshape(B, S, HID)
